# revision 3
# baseline (speedup 1.0000x reference)
"""Trainium2 Bass kernel for the ButterflyMlp problem.

Computes log_softmax(L3(relu(L2(relu(L1(x)))))) where each Li is a masked
linear layer (butterfly sparsity: global column stripes + a diagonal band),
batch 65536, data-parallel over 8 NeuronCores (8192 rows/core).

Strategy (per core, feature-major throughout):
  - Masks are pre-applied to weights on host. Layer-1 exploits the butterfly
    structure: the stripe columns (mask true for every output row) form a
    dense [|S|, 784] GEMM shared by all outputs, and the per-output-block
    band adds one narrow [|R_j|<=128, 112] GEMM per 112-row output block.
    This cuts layer-1 matmul passes from 49 to 21 per batch chunk.
  - All GEMMs run in float32r (~1.6e-4 relative error, 2x the bf16 cycle
    cost but 4x better than plain fp32 on the PE).
  - x rows are pre-gathered on host (stripe rows + per-block band rows) so
    every DMA is a dense 2D block; loads are issued in 1024-column
    superchunks alternating between the two HWDGE rings (sync/scalar) so
    the two rings drain concurrently.
  - ReLU+bias fuses into the PSUM->SBUF eviction, alternating ScalarE and
    VectorE.
  - log_softmax stays feature-major: lse via exp (ACT) -> ones-matmul
    column-sum (PE) -> ln (ACT) -> ones-matmul partition-broadcast (PE) ->
    subtract (DVE). No max subtraction needed (logits are O(1); exp is safe).
  - Output is [10, 8192] per core; host transposes and concatenates.
"""
import sys
sys.path.insert(0, "/opt/trn_rl_repo")
import numpy as np

import concourse.bass as bass
import concourse.bacc as bacc
import concourse.mybir as mybir
import concourse.tile as tile
from concourse import bass_utils

F32 = mybir.dt.float32
F32R = mybir.dt.float32r
AF = mybir.ActivationFunctionType
ALU = mybir.AluOpType

N_CORES = 8
NB = 512          # batch columns per matmul (one PSUM bank of fp32)
SC = 1024         # batch columns per DMA superchunk
OT = 112          # layer-1 output block width (784/7; band window fits 128)


def _decompose_mask1(mask1):
    """Split the butterfly mask into stripe columns S (true for every row)
    and per-output-block residual columns R_j."""
    D_out, D_in = mask1.shape
    S = np.where(mask1.all(axis=0))[0]
    n_blk = (D_out + OT - 1) // OT
    stripe_set = np.zeros(D_in, dtype=bool)
    stripe_set[S] = True
    R_list = []
    for j in range(n_blk):
        blk = mask1[j * OT:(j + 1) * OT]
        cols = np.where(blk.any(axis=0) & ~stripe_set)[0]
        assert len(cols) <= 128, f"band block {j} has {len(cols)} cols"
        R_list.append(cols)
    return S, R_list


def _build_program(meta):
    nS, nR_tot, R_lens = meta["nS"], meta["nR_tot"], meta["R_lens"]
    Bc = meta["Bc"]
    D1, H, C = meta["D1"], meta["H"], meta["C"]
    n_blk = len(R_lens)
    n_sc = (nS + 127) // 128              # stripe K-chunks
    sc_sizes = [nS // n_sc + (1 if i < nS % n_sc else 0) for i in range(n_sc)]
    sc_off = np.cumsum([0] + sc_sizes)
    n_kc2 = D1 // OT                      # layer-2 K chunks (= n_blk)
    n_sup = Bc // SC                      # DMA superchunks
    n_half = SC // NB                     # matmul chunks per superchunk
    R_off = np.cumsum([0] + R_lens)

    nc = bacc.Bacc("TRN2", target_bir_lowering=False, debug=False,
                   enable_asserts=False, num_devices=N_CORES)

    xs_d = nc.dram_tensor("xs", [nS, Bc], F32R, kind="ExternalInput").ap()
    xb_d = nc.dram_tensor("xb", [nR_tot, Bc], F32R, kind="ExternalInput").ap()
    ws_d = nc.dram_tensor("ws", [nS, D1], F32R, kind="ExternalInput").ap()
    wb_d = nc.dram_tensor("wb", [128, n_blk * OT], F32R, kind="ExternalInput").ap()
    w2_d = nc.dram_tensor("w2", [OT, n_kc2 * H], F32R, kind="ExternalInput").ap()
    w3_d = nc.dram_tensor("w3", [H, C], F32R, kind="ExternalInput").ap()
    b1_d = nc.dram_tensor("b1", [OT, n_blk], F32, kind="ExternalInput").ap()
    b2_d = nc.dram_tensor("b2", [H, 1], F32, kind="ExternalInput").ap()
    b3_d = nc.dram_tensor("b3", [C, 1], F32, kind="ExternalInput").ap()
    onc_d = nc.dram_tensor("onc", [C, 1], F32R, kind="ExternalInput").ap()
    onr_d = nc.dram_tensor("onr", [1, C], F32R, kind="ExternalInput").ap()
    out_d = nc.dram_tensor("out", [C, Bc], F32, kind="ExternalOutput").ap()

    # round-robin across the two HWDGE rings
    rr = [0]
    def dma(dst, src):
        eng = nc.sync if rr[0] % 2 == 0 else nc.scalar
        rr[0] += 1
        eng.dma_start(dst, src)

    with tile.TileContext(nc) as tc:
        with tc.tile_pool(name="wp", bufs=1) as wp, \
             tc.tile_pool(name="xp", bufs=3) as xp, \
             tc.tile_pool(name="hp", bufs=2) as hp, \
             tc.tile_pool(name="op", bufs=2) as op, \
             tc.tile_pool(name="ps1", bufs=3, space="PSUM") as ps1, \
             tc.tile_pool(name="ps2", bufs=2, space="PSUM") as ps2, \
             tc.tile_pool(name="ps3", bufs=1, space="PSUM") as ps3, \
             tc.tile_pool(name="ps4", bufs=1, space="PSUM") as ps4, \
             tc.tile_pool(name="ps5", bufs=1, space="PSUM") as ps5:

            # ---- resident weights (interleaved with first x superchunk) ----
            ws_sb = []
            for c in range(n_sc):
                t = wp.tile([sc_sizes[c], D1], F32R, name=f"ws_sb{c}")
                nc.scalar.dma_start(t[:], ws_d[sc_off[c]:sc_off[c + 1], :])
                ws_sb.append(t)
            wb_sb = wp.tile([128, n_blk * OT], F32R)
            nc.scalar.dma_start(wb_sb[:], wb_d[:])
            w2_sb = wp.tile([OT, n_kc2 * H], F32R)
            nc.sync.dma_start(w2_sb[:], w2_d[:])
            w3_sb = wp.tile([H, C], F32R)
            nc.sync.dma_start(w3_sb[:], w3_d[:])
            b1_sb = wp.tile([OT, n_blk], F32)
            nc.sync.dma_start(b1_sb[:], b1_d[:])
            b2_sb = wp.tile([H, 1], F32)
            nc.sync.dma_start(b2_sb[:], b2_d[:])
            b3_sb = wp.tile([C, 1], F32)
            nc.sync.dma_start(b3_sb[:], b3_d[:])
            onc_sb = wp.tile([C, 1], F32R)
            nc.sync.dma_start(onc_sb[:], onc_d[:])
            onr_sb = wp.tile([1, C], F32R)
            nc.sync.dma_start(onr_sb[:], onr_d[:])

            for s in range(n_sup):
                ss = s * SC
                # ---- x loads: one superchunk, both rings ----
                xs_t = []
                for c in range(n_sc):
                    t = xp.tile([sc_sizes[c], SC], F32R, name=f"xs_t{c}",
                                tag=f"xs{c}")
                    dma(t[:], xs_d[sc_off[c]:sc_off[c + 1], ss:ss + SC])
                    xs_t.append(t)
                xb_t = []
                for j in range(n_blk):
                    t = xp.tile([R_lens[j], SC], F32R, name=f"xb_t{j}",
                                tag=f"xb{j}")
                    dma(t[:], xb_d[R_off[j]:R_off[j + 1], ss:ss + SC])
                    xb_t.append(t)

                for h2 in range(n_half):
                    hs = h2 * NB
                    bs = ss + hs
                    # ---- layer 1 ----
                    y1_t = []
                    for j in range(n_blk):
                        p = ps1.tile([OT, NB], F32, tag="l1")
                        for c in range(n_sc):
                            nc.tensor.matmul(
                                p[:], ws_sb[c][:, j * OT:(j + 1) * OT],
                                xs_t[c][:, hs:hs + NB],
                                start=(c == 0), stop=False)
                        nc.tensor.matmul(
                            p[:], wb_sb[:R_lens[j], j * OT:(j + 1) * OT],
                            xb_t[j][:, hs:hs + NB], start=False, stop=True)
                        h = hp.tile([OT, NB], F32R, name=f"y1_{j}", tag=f"y1{j}")
                        if j % 2 == 0:
                            nc.vector.tensor_scalar(h[:], p[:],
                                                    b1_sb[:, j:j + 1], 0.0,
                                                    op0=ALU.add, op1=ALU.max)
                        else:
                            nc.scalar.activation(h[:], p[:], AF.Relu,
                                                 bias=b1_sb[:, j:j + 1])
                        y1_t.append(h)

                    # ---- layer 2 ----
                    p2 = ps2.tile([H, NB], F32, tag="l2")
                    for k in range(n_kc2):
                        nc.tensor.matmul(p2[:], w2_sb[:, k * H:(k + 1) * H],
                                         y1_t[k][:], start=(k == 0),
                                         stop=(k == n_kc2 - 1))
                    y2 = hp.tile([H, NB], F32R, tag="y2")
                    nc.scalar.activation(y2[:], p2[:], AF.Relu,
                                         bias=b2_sb[:, 0:1])

                    # ---- layer 3 + feature-major log_softmax ----
                    p3 = ps3.tile([C, NB], F32, tag="l3")
                    nc.tensor.matmul(p3[:], w3_sb[:], y2[:], start=True,
                                     stop=True)
                    y3t = hp.tile([C, NB], F32R, tag="y3t")
                    nc.scalar.activation(y3t[:], p3[:], AF.Identity,
                                         bias=b3_sb[:, 0:1])
                    ex = hp.tile([C, NB], F32R, tag="ex")
                    nc.scalar.activation(ex[:], y3t[:], AF.Exp)
                    p_s = ps4.tile([1, NB], F32, tag="lsum")
                    nc.tensor.matmul(p_s[:], onc_sb[:], ex[:], start=True,
                                     stop=True)
                    ls = hp.tile([1, NB], F32R, tag="ls")
                    nc.scalar.activation(ls[:], p_s[:], AF.Ln)
                    p_bc = ps5.tile([C, NB], F32, tag="lbc")
                    nc.tensor.matmul(p_bc[:], onr_sb[:], ls[:], start=True,
                                     stop=True)
                    o = op.tile([C, NB], F32, tag="o")
                    nc.vector.tensor_tensor(o[:], y3t[:].bitcast(F32), p_bc[:],
                                            op=ALU.subtract)
                    nc.gpsimd.dma_start(out_d[:, bs:bs + NB], o[:])

    nc.compile()
    return nc


_CACHE = {}


def _prepare(x, W1, b1, W2, b2, W3, b3, mask1, mask2, mask3):
    B, D1 = x.shape
    H = W2.shape[0]
    C = W3.shape[0]
    assert B % N_CORES == 0
    Bc = B // N_CORES

    S, R_list = _decompose_mask1(np.asarray(mask1))
    R_lens = [len(r) for r in R_list]
    n_blk = len(R_list)

    Wm1 = (np.asarray(W1) * np.asarray(mask1)).astype(np.float32)
    Wm2 = (np.asarray(W2) * np.asarray(mask2)).astype(np.float32)
    Wm3 = (np.asarray(W3) * np.asarray(mask3)).astype(np.float32)

    ws = np.ascontiguousarray(Wm1[:, S].T)                # [|S|, D1]
    wb = np.zeros((128, n_blk * OT), np.float32)
    for j, R in enumerate(R_list):
        wb[:len(R), j * OT:j * OT + OT] = Wm1[j * OT:(j + 1) * OT, R].T
    n_kc2 = D1 // OT
    w2 = np.ascontiguousarray(
        Wm2.T.reshape(n_kc2, OT, H).transpose(1, 0, 2).reshape(OT, n_kc2 * H))
    w3 = np.ascontiguousarray(Wm3.T)                      # [H, C]
    b1p = np.ascontiguousarray(
        np.asarray(b1, np.float32).reshape(n_blk, OT).T)  # [OT, n_blk]
    b2p = np.asarray(b2, np.float32).reshape(H, 1)
    b3p = np.asarray(b3, np.float32).reshape(C, 1)

    xT = np.asarray(x, np.float32).T                      # [D1, B] view
    xs_all = np.ascontiguousarray(xT[S])                  # [|S|, B]
    R_cat = np.concatenate(R_list)
    xb_all = np.ascontiguousarray(xT[R_cat])              # [sum R, B]

    meta = dict(nS=len(S), nR_tot=len(R_cat), R_lens=R_lens,
                Bc=Bc, D1=D1, H=H, C=C)
    key = (B, D1, H, C, len(S), tuple(R_lens))
    if key not in _CACHE:
        _CACHE[key] = _build_program(meta)
    nc = _CACHE[key]

    in_maps = []
    for c in range(N_CORES):
        sl = slice(c * Bc, (c + 1) * Bc)
        in_maps.append({
            "xs": np.ascontiguousarray(xs_all[:, sl]),
            "xb": np.ascontiguousarray(xb_all[:, sl]),
            "ws": ws, "wb": wb, "w2": w2, "w3": w3,
            "b1": b1p, "b2": b2p, "b3": b3p,
            "onc": np.ones((C, 1), np.float32),
            "onr": np.ones((1, C), np.float32),
        })
    return nc, in_maps, meta


def _assemble(results, meta):
    C = meta["C"]
    outs = [np.ascontiguousarray(results[c]["out"].T)     # [Bc, C]
            for c in range(N_CORES)]
    return np.concatenate(outs, axis=0).astype(np.float32)


def kernel(**inputs):
    nc, in_maps, meta = _prepare(**inputs)
    res = bass_utils.run_bass_kernel_spmd(nc, in_maps,
                                          core_ids=list(range(N_CORES)))
    return _assemble(res.results, meta)


def kernel_traced(tmpdir=None, **inputs):
    """Same as kernel() but with NTFF profiling; returns (output, results)."""
    nc, in_maps, meta = _prepare(**inputs)
    res = bass_utils.run_bass_kernel_spmd(nc, in_maps,
                                          core_ids=list(range(N_CORES)),
                                          trace=True, tmpdir=tmpdir)
    return _assemble(res.results, meta), res
